# revision 14
# baseline (speedup 1.0000x reference)
"""HQQ-quantized linear + LoRA (nn_HQQLinearLoRA) on 8 trn2 NeuronCores.

  out = x @ ((W_q - zero)*scale)^T + (x @ lora_A @ lora_B) * 2.0 + bias

Sharding: 4 token-groups (batch dim) x 2 out-feature-groups = 8 cores.
Each core computes out[b, :, og*2048:(og+1)*2048].

Key structure (per core):
  - zero-point is FOLDED OUT of the per-element dequant:
        W_deq = W_q*scale - zero*scale
    so device dequant is a single multiply, and the -zero*scale term
    becomes part of the fused rank-81 "appendix" matmul:
        out_tile += [t1; y; ones]^T @ [lora_B; -(zero*scale)^T; bias]
    where t1 = x @ (2*lora_A) (16 rows) and y[t,g] = sum_{i in g} x[t,i]
    (64 group-sum rows) are produced by one fused matmul per k-tile
    against a [128, 80] combined operand (lora_A columns + 0/1 group
    masks -- constant across k-tiles thanks to the row permutation).
  - W (fp16, host-pre-cast from int 0..15) stays fully resident in SBUF;
    x streams in 8 token chunks; all DMAs contiguous HWDGE.
  - The first two chunks interleave o-quarters in arrival order of W so
    the PE never waits long during the W load/dequant ramp.

Host passes pre-transposed/pre-cast fp16 views (layout/dtype-only):
  xh [NTCH, 128, KT*TCH], wh [NOCH, 128, KT*OCH] (rows permuted so
  partition p has quant-group g = p%64), s16 [128, O_SH] (scale rows
  pre-expanded), la80 [128, KT*80], lb [81, O_SH], ones [1, T].
"""

import sys

import numpy as np

sys.path.append("/opt/trn_rl_repo")

import concourse.bass as bass  # noqa: E402
import concourse.mybir as mybir  # noqa: E402
import concourse.tile as tile  # noqa: E402
from concourse import bacc  # noqa: E402
from concourse.bass_utils import run_bass_kernel_spmd  # noqa: E402

B, S, I, O, R = 4, 2048, 4096, 4096, 16
GS = 64
G = I // GS  # 64
NCORES = 8
OG = 2
O_SH = O // OG  # 2048
T = S  # 2048 tokens per core
KT = I // 128  # 32 k-tiles
TCH = 256  # token chunk
NTCH = T // TCH  # 8
OCH = 512  # o-quarter (one PSUM bank wide)
NOCH = O_SH // OCH  # 4
WSUB = 4  # k-tiles per W load sub-block
NSUB = KT // WSUB  # 8
AUG = R + G + 1  # 81 appendix rows: [t1(16); y(64); ones(1)]
AUG1 = AUG - 1  # 80 columns in the fused t1/y matmul
SCALING = 2.0

F32 = mybir.dt.float32
F16 = mybir.dt.float16

TRACE = False
TRACE_KWARGS = {}
TRACE_TMPDIR = None
LAST_RESULTS = None


def _perm() -> np.ndarray:
    """Row order such that k-tile k, partition p holds input-feature
    i = (p % 64)*64 + 2k + p//64, i.e. quant group g(i) = p % 64."""
    p = np.arange(128)
    out = np.empty(I, dtype=np.int64)
    for k in range(KT):
        out[k * 128 + p] = (p % 64) * 64 + 2 * k + p // 64
    return out


PERM = _perm()

_nc_cache = None


def _build():
    nc = bacc.Bacc(None)
    xh_d = nc.dram_tensor("xh", [NTCH, 128, KT * TCH], F16, kind="ExternalInput")
    wh_d = nc.dram_tensor("wh", [NOCH, 128, KT * OCH], F16, kind="ExternalInput")
    s_d = nc.dram_tensor("s16", [128, O_SH], F16, kind="ExternalInput")
    la_d = nc.dram_tensor("la80", [128, KT * AUG1], F16, kind="ExternalInput")
    lb_d = nc.dram_tensor("lb", [AUG, O_SH], F16, kind="ExternalInput")
    ones_d = nc.dram_tensor("ones", [1, T], F16, kind="ExternalInput")
    out_d = nc.dram_tensor("out", [T, O_SH], F32, kind="ExternalOutput")

    Copy = mybir.ActivationFunctionType.Copy

    with tile.TileContext(nc) as tc:
        with (
            tc.tile_pool(name="const", bufs=1) as constp,
            tc.tile_pool(name="w16", bufs=NOCH) as wp,
            tc.tile_pool(name="ws", bufs=2) as wsp,
            tc.tile_pool(name="x16", bufs=3) as xp,
            tc.tile_pool(name="ob", bufs=3) as obp,
            tc.tile_pool(name="ps", bufs=6, space="PSUM") as psp,
            tc.tile_pool(name="psl", bufs=2, space="PSUM") as pslp,
        ):
            # ---- constants ----
            s16 = constp.tile([128, O_SH], F16)
            la80 = constp.tile([128, KT, AUG1], F16)
            lb16 = constp.tile([AUG, O_SH], F16)
            t1sb = constp.tile([AUG, T], F16)
            nc.sync.dma_start(s16[:], s_d[:, :])
            nc.sync.dma_start(la80[:], la_d.rearrange("p (k r) -> p k r", k=KT))
            nc.sync.dma_start(lb16[:], lb_d[:, :])
            nc.sync.dma_start(t1sb[AUG - 1 : AUG, :], ones_d[:])

            # ---- x chunk loads (prefetched ahead of use) ----
            xtiles = {}

            def load_x(c):
                t = xp.tile([128, KT, TCH], F16)
                nc.sync.dma_start(t[:], xh_d[c].rearrange("p (k t) -> p k t", k=KT))
                xtiles[c] = t

            # ---- W quarters: resident tiles; sub-block loads + 1-mul dequant
            w16 = [
                wp.tile([128, KT, OCH], F16, name=f"w{q}", tag="w")
                for q in range(NOCH)
            ]

            def load_w(q):
                for j in range(NSUB):
                    ws = wsp.tile([128, WSUB, OCH], F16)
                    nc.sync.dma_start(
                        ws[:],
                        wh_d[q].rearrange("p (k o) -> p k o", k=KT)[
                            :, j * WSUB : (j + 1) * WSUB, :
                        ],
                    )
                    for kk in range(WSUB):
                        k = j * WSUB + kk
                        nc.vector.tensor_mul(
                            w16[q][:, k, :],
                            ws[:, kk, :],
                            s16[:, q * OCH : (q + 1) * OCH],
                        )

            # DMA order: consts, x0-x2, then W quarters (x feeds the t1
            # warmup passes that fill the PE while W streams in)
            load_x(0)
            load_x(1)
            load_x(2)
            load_w(0)
            load_w(1)

            def t1_pass(c):
                x16 = xtiles[c]
                t1ps = pslp.tile([AUG - 1, TCH], F32)
                for k in range(KT):
                    nc.tensor.matmul(
                        t1ps[:],
                        la80[:, k, :],
                        x16[:, k, :],
                        start=(k == 0),
                        stop=(k == KT - 1),
                    )
                nc.scalar.activation(
                    t1sb[0 : AUG - 1, c * TCH : (c + 1) * TCH], t1ps[:], Copy
                )

            def pair(c, q):
                x16 = xtiles[c]
                for tt in range(TCH // 128):
                    t0 = c * TCH + tt * 128
                    ps = psp.tile([128, OCH], F32)
                    for k in range(KT):
                        nc.tensor.matmul(
                            ps[:],
                            x16[:, k, tt * 128 : tt * 128 + 128],
                            w16[q][:, k, :],
                            start=(k == 0),
                            stop=False,
                        )
                    nc.tensor.matmul(
                        ps[:],
                        t1sb[:, t0 : t0 + 128],
                        lb16[:, q * OCH : (q + 1) * OCH],
                        start=False,
                        stop=True,
                    )
                    ob = obp.tile([128, OCH], F32)
                    nc.vector.tensor_copy(ob[:], ps[:])
                    nc.scalar.dma_start(
                        out_d[t0 : t0 + 128, q * OCH : (q + 1) * OCH], ob[:]
                    )

            # ---- staggered warmup: chunks 0-2 follow W quarter arrival ----
            t1_pass(0)
            t1_pass(1)
            t1_pass(2)
            pair(0, 0)
            pair(1, 0)
            load_w(2)
            pair(2, 0)
            pair(0, 1)
            load_w(3)
            pair(1, 1)
            pair(2, 1)
            pair(0, 2)
            pair(1, 2)
            pair(2, 2)
            pair(0, 3)
            load_x(3)
            pair(1, 3)
            pair(2, 3)
            xtiles.pop(0)
            xtiles.pop(1)
            xtiles.pop(2)

            # ---- steady state ----
            for c in range(3, NTCH):
                if c + 1 < NTCH:
                    load_x(c + 1)
                t1_pass(c)
                for q in range(NOCH):
                    pair(c, q)
                xtiles.pop(c)

    nc.compile()
    return nc


def kernel(x, W_q, scale, zero, lora_A, lora_B, bias):
    global _nc_cache, LAST_RESULTS
    if _nc_cache is None:
        _nc_cache = _build()
    nc = _nc_cache

    x = np.asarray(x, dtype=np.float32)
    W_q = np.asarray(W_q, dtype=np.int32)
    scale = np.asarray(scale, dtype=np.float32)
    zero = np.asarray(zero, dtype=np.float32)
    lora_A = np.asarray(lora_A, dtype=np.float32)
    lora_B = np.asarray(lora_B, dtype=np.float32)
    bias = np.asarray(bias, dtype=np.float32)

    # combined [t1 | y] operand: 2*lora_A columns + 0/1 group masks
    laf = (lora_A[PERM] * SCALING).astype(np.float16)  # [I, R]
    la3 = laf.reshape(KT, 128, R)
    mask = (np.arange(128)[:, None] % G == np.arange(G)[None, :]).astype(np.float16)
    la80 = np.empty((128, KT, AUG1), dtype=np.float16)
    la80[:, :, :R] = la3.transpose(1, 0, 2)
    la80[:, :, R:] = mask[:, None, :]
    la_h = np.ascontiguousarray(la80).reshape(128, KT * AUG1)
    ones = np.ones((1, T), dtype=np.float16)

    # x per batch element (shared by the 2 o-group cores)
    xh_b = []
    for b in range(B):
        xt = x[b].T[PERM].astype(np.float16)  # [I, T]
        xh = np.ascontiguousarray(
            xt.reshape(KT, 128, NTCH, TCH).transpose(2, 1, 0, 3)
        ).reshape(NTCH, 128, KT * TCH)
        xh_b.append(xh)

    in_maps = []
    for c in range(NCORES):
        b, og = c // OG, c % OG
        osl = slice(og * O_SH, (og + 1) * O_SH)
        wt = W_q[osl].T[PERM].astype(np.float16)  # [I, O_SH]
        wh = np.ascontiguousarray(
            wt.reshape(KT, 128, NOCH, OCH).transpose(2, 1, 0, 3)
        ).reshape(NOCH, 128, KT * OCH)
        st = scale[osl].T.astype(np.float16)  # [G, O_SH]
        zs = -(zero[osl] * scale[osl]).T.astype(np.float16)  # [G, O_SH]
        lb = np.empty((AUG, O_SH), dtype=np.float16)
        lb[:R] = lora_B[:, osl].astype(np.float16)
        lb[R : R + G] = zs
        lb[AUG - 1] = bias[osl].astype(np.float16)
        in_maps.append(
            {
                "xh": xh_b[b],
                "wh": wh,
                "s16": np.ascontiguousarray(np.concatenate([st, st], axis=0)),
                "la80": la_h,
                "lb": lb,
                "ones": ones,
            }
        )

    res = run_bass_kernel_spmd(
        nc,
        in_maps,
        core_ids=list(range(NCORES)),
        trace=TRACE,
        trace_kwargs=TRACE_KWARGS,
        tmpdir=TRACE_TMPDIR,
    )
    LAST_RESULTS = res

    out = np.empty((B, S, O), dtype=np.float32)
    for c in range(NCORES):
        b, og = c // OG, c % OG
        out[b, :, og * O_SH : (og + 1) * O_SH] = res.results[c]["out"]
    return out


# revision 17
# speedup vs baseline: 1.0115x; 1.0115x over previous
"""HQQ-quantized linear + LoRA (nn_HQQLinearLoRA) on 8 trn2 NeuronCores.

  out = x @ ((W_q - zero)*scale)^T + (x @ lora_A @ lora_B) * 2.0 + bias

Sharding: 4 token-groups (batch dim) x 2 out-feature-groups = 8 cores.
Each core computes out[b, :, og*2048:(og+1)*2048].

Key structure (per core):
  - zero-point is FOLDED OUT of the per-element dequant:
        W_deq = W_q*scale - zero*scale
    so device dequant is a single multiply, and the -zero*scale term
    becomes part of the fused rank-81 "appendix" matmul:
        out_tile += [t1; y; ones]^T @ [lora_B; -(zero*scale)^T; bias]
    where t1 = x @ (2*lora_A) (16 rows) and y[t,g] = sum_{i in g} x[t,i]
    (64 group-sum rows) are produced by one fused matmul per k-tile
    against a [128, 80] combined operand (lora_A columns + 0/1 group
    masks -- constant across k-tiles thanks to the row permutation).
  - W (fp16, host-pre-cast from int 0..15) stays fully resident in SBUF;
    x streams in 8 token chunks; all DMAs contiguous HWDGE.
  - The first two chunks interleave o-quarters in arrival order of W so
    the PE never waits long during the W load/dequant ramp.

Host passes pre-transposed/pre-cast fp16 views (layout/dtype-only):
  xh [NTCH, 128, KT*TCH], wh [NOCH, 128, KT*OCH] (rows permuted so
  partition p has quant-group g = p%64), s16 [128, O_SH] (scale rows
  pre-expanded), la80 [128, KT*80], lb [81, O_SH], ones [1, T].
"""

import sys

import numpy as np

sys.path.append("/opt/trn_rl_repo")

import concourse.bass as bass  # noqa: E402
import concourse.mybir as mybir  # noqa: E402
import concourse.tile as tile  # noqa: E402
from concourse import bacc  # noqa: E402
from concourse.bass_utils import run_bass_kernel_spmd  # noqa: E402

B, S, I, O, R = 4, 2048, 4096, 4096, 16
GS = 64
G = I // GS  # 64
NCORES = 8
OG = 2
O_SH = O // OG  # 2048
T = S  # 2048 tokens per core
KT = I // 128  # 32 k-tiles
TCH = 256  # token chunk
NTCH = T // TCH  # 8
OCH = 512  # o-quarter (one PSUM bank wide)
NOCH = O_SH // OCH  # 4
WSUB = 4  # k-tiles per W load sub-block
NSUB = KT // WSUB  # 8
AUG = R + G + 1  # 81 appendix rows: [t1(16); y(64); ones(1)]
AUG1 = AUG - 1  # 80 columns in the fused t1/y matmul
SCALING = 2.0

F32 = mybir.dt.float32
F16 = mybir.dt.float16

TRACE = False
TRACE_KWARGS = {}
TRACE_TMPDIR = None
LAST_RESULTS = None


def _perm() -> np.ndarray:
    """Row order such that k-tile k, partition p holds input-feature
    i = (p % 64)*64 + 2k + p//64, i.e. quant group g(i) = p % 64."""
    p = np.arange(128)
    out = np.empty(I, dtype=np.int64)
    for k in range(KT):
        out[k * 128 + p] = (p % 64) * 64 + 2 * k + p // 64
    return out


PERM = _perm()

_nc_cache = None


def _build():
    nc = bacc.Bacc(None)
    xh_d = nc.dram_tensor("xh", [NTCH, 128, KT * TCH], F16, kind="ExternalInput")
    wh_d = nc.dram_tensor("wh", [NOCH, 128, KT * OCH], F16, kind="ExternalInput")
    s_d = nc.dram_tensor("s16", [128, O_SH], F16, kind="ExternalInput")
    la_d = nc.dram_tensor("la80", [128, KT * AUG1], F16, kind="ExternalInput")
    lb_d = nc.dram_tensor("lb", [AUG, O_SH], F16, kind="ExternalInput")
    ones_d = nc.dram_tensor("ones", [1, T], F16, kind="ExternalInput")
    out_d = nc.dram_tensor("out", [T, O_SH], F32, kind="ExternalOutput")

    Copy = mybir.ActivationFunctionType.Copy

    with tile.TileContext(nc) as tc:
        with (
            tc.tile_pool(name="const", bufs=1) as constp,
            tc.tile_pool(name="w16", bufs=NOCH) as wp,
            tc.tile_pool(name="ws", bufs=2) as wsp,
            tc.tile_pool(name="x16", bufs=3) as xp,
            tc.tile_pool(name="ob", bufs=3) as obp,
            tc.tile_pool(name="ps", bufs=6, space="PSUM") as psp,
            tc.tile_pool(name="psl", bufs=2, space="PSUM") as pslp,
        ):
            # ---- constants ----
            s16 = constp.tile([128, O_SH], F16)
            la80 = constp.tile([128, KT, AUG1], F16)
            lb16 = constp.tile([AUG, O_SH], F16)
            t1sb = constp.tile([AUG, T], F16)
            nc.sync.dma_start(s16[:], s_d[:, :])
            nc.sync.dma_start(la80[:], la_d.rearrange("p (k r) -> p k r", k=KT))
            nc.sync.dma_start(lb16[:], lb_d[:, :])
            nc.sync.dma_start(t1sb[AUG - 1 : AUG, :], ones_d[:])

            # ---- x chunk loads (prefetched ahead of use) ----
            xtiles = {}

            def load_x(c):
                t = xp.tile([128, KT, TCH], F16)
                nc.sync.dma_start(t[:], xh_d[c].rearrange("p (k t) -> p k t", k=KT))
                xtiles[c] = t

            # ---- W quarters: resident tiles; sub-block loads + 1-mul dequant
            w16 = [
                wp.tile([128, KT, OCH], F16, name=f"w{q}", tag="w")
                for q in range(NOCH)
            ]

            def load_w(q, jlo=0, jhi=NSUB):
                for j in range(jlo, jhi):
                    ws = wsp.tile([128, WSUB, OCH], F16)
                    nc.sync.dma_start(
                        ws[:],
                        wh_d[q].rearrange("p (k o) -> p k o", k=KT)[
                            :, j * WSUB : (j + 1) * WSUB, :
                        ],
                    )
                    for kk in range(WSUB):
                        k = j * WSUB + kk
                        nc.vector.tensor_mul(
                            w16[q][:, k, :],
                            ws[:, kk, :],
                            s16[:, q * OCH : (q + 1) * OCH],
                        )

            # DMA order: la80/s16 (t1 + dequant deps), x0-x1, first half of
            # W quarter 0, x2, second half -- so main matmuls can start on
            # half-K accumulation while the rest of W streams in
            load_x(0)
            load_x(1)
            load_w(0, 0, NSUB // 2)
            load_x(2)
            load_w(0, NSUB // 2, NSUB)

            def t1_pass(c):
                x16 = xtiles[c]
                t1ps = pslp.tile([AUG - 1, TCH], F32)
                for k in range(KT):
                    nc.tensor.matmul(
                        t1ps[:],
                        la80[:, k, :],
                        x16[:, k, :],
                        start=(k == 0),
                        stop=(k == KT - 1),
                    )
                nc.scalar.activation(
                    t1sb[0 : AUG - 1, c * TCH : (c + 1) * TCH], t1ps[:], Copy
                )

            def pair_start(c, q, khi=KT):
                x16 = xtiles[c]
                pss = []
                for tt in range(TCH // 128):
                    ps = psp.tile([128, OCH], F32)
                    for k in range(khi):
                        nc.tensor.matmul(
                            ps[:],
                            x16[:, k, tt * 128 : tt * 128 + 128],
                            w16[q][:, k, :],
                            start=(k == 0),
                            stop=False,
                        )
                    pss.append(ps)
                return pss

            def pair_finish(c, q, pss, klo=KT):
                x16 = xtiles[c]
                for tt in range(TCH // 128):
                    t0 = c * TCH + tt * 128
                    ps = pss[tt]
                    for k in range(klo, KT):
                        nc.tensor.matmul(
                            ps[:],
                            x16[:, k, tt * 128 : tt * 128 + 128],
                            w16[q][:, k, :],
                            start=False,
                            stop=False,
                        )
                    nc.tensor.matmul(
                        ps[:],
                        t1sb[:, t0 : t0 + 128],
                        lb16[:, q * OCH : (q + 1) * OCH],
                        start=False,
                        stop=True,
                    )
                    ob = obp.tile([128, OCH], F32)
                    nc.vector.tensor_copy(ob[:], ps[:])
                    nc.scalar.dma_start(
                        out_d[t0 : t0 + 128, q * OCH : (q + 1) * OCH], ob[:]
                    )

            def pair(c, q):
                pair_finish(c, q, pair_start(c, q))

            # ---- staggered warmup: chunks 0-2 follow W arrival; the first
            # two (c, q=0) pairs split their K-accumulation so matmuls can
            # begin on the first half of quarter 0 ----
            KH = KT // 2
            t1_pass(0)
            t1_pass(1)
            ps00 = pair_start(0, 0, khi=KH)
            load_w(1)
            ps10 = pair_start(1, 0, khi=KH)
            t1_pass(2)
            pair_finish(0, 0, ps00, klo=KH)
            load_w(2)
            pair_finish(1, 0, ps10, klo=KH)
            pair(2, 0)
            pair(0, 1)
            load_w(3)
            pair(1, 1)
            pair(2, 1)
            pair(0, 2)
            pair(1, 2)
            pair(2, 2)
            pair(0, 3)
            load_x(3)
            pair(1, 3)
            pair(2, 3)
            xtiles.pop(0)
            xtiles.pop(1)
            xtiles.pop(2)

            # ---- steady state ----
            for c in range(3, NTCH):
                if c + 1 < NTCH:
                    load_x(c + 1)
                t1_pass(c)
                for q in range(NOCH):
                    pair(c, q)
                xtiles.pop(c)

    nc.compile()
    return nc


def kernel(x, W_q, scale, zero, lora_A, lora_B, bias):
    global _nc_cache, LAST_RESULTS
    if _nc_cache is None:
        _nc_cache = _build()
    nc = _nc_cache

    x = np.asarray(x, dtype=np.float32)
    W_q = np.asarray(W_q, dtype=np.int32)
    scale = np.asarray(scale, dtype=np.float32)
    zero = np.asarray(zero, dtype=np.float32)
    lora_A = np.asarray(lora_A, dtype=np.float32)
    lora_B = np.asarray(lora_B, dtype=np.float32)
    bias = np.asarray(bias, dtype=np.float32)

    # combined [t1 | y] operand: 2*lora_A columns + 0/1 group masks
    laf = (lora_A[PERM] * SCALING).astype(np.float16)  # [I, R]
    la3 = laf.reshape(KT, 128, R)
    mask = (np.arange(128)[:, None] % G == np.arange(G)[None, :]).astype(np.float16)
    la80 = np.empty((128, KT, AUG1), dtype=np.float16)
    la80[:, :, :R] = la3.transpose(1, 0, 2)
    la80[:, :, R:] = mask[:, None, :]
    la_h = np.ascontiguousarray(la80).reshape(128, KT * AUG1)
    ones = np.ones((1, T), dtype=np.float16)

    # x per batch element (shared by the 2 o-group cores)
    xh_b = []
    for b in range(B):
        xt = x[b].T[PERM].astype(np.float16)  # [I, T]
        xh = np.ascontiguousarray(
            xt.reshape(KT, 128, NTCH, TCH).transpose(2, 1, 0, 3)
        ).reshape(NTCH, 128, KT * TCH)
        xh_b.append(xh)

    in_maps = []
    for c in range(NCORES):
        b, og = c // OG, c % OG
        osl = slice(og * O_SH, (og + 1) * O_SH)
        wt = W_q[osl].T[PERM].astype(np.float16)  # [I, O_SH]
        wh = np.ascontiguousarray(
            wt.reshape(KT, 128, NOCH, OCH).transpose(2, 1, 0, 3)
        ).reshape(NOCH, 128, KT * OCH)
        st = scale[osl].T.astype(np.float16)  # [G, O_SH]
        zs = -(zero[osl] * scale[osl]).T.astype(np.float16)  # [G, O_SH]
        lb = np.empty((AUG, O_SH), dtype=np.float16)
        lb[:R] = lora_B[:, osl].astype(np.float16)
        lb[R : R + G] = zs
        lb[AUG - 1] = bias[osl].astype(np.float16)
        in_maps.append(
            {
                "xh": xh_b[b],
                "wh": wh,
                "s16": np.ascontiguousarray(np.concatenate([st, st], axis=0)),
                "la80": la_h,
                "lb": lb,
                "ones": ones,
            }
        )

    res = run_bass_kernel_spmd(
        nc,
        in_maps,
        core_ids=list(range(NCORES)),
        trace=TRACE,
        trace_kwargs=TRACE_KWARGS,
        tmpdir=TRACE_TMPDIR,
    )
    LAST_RESULTS = res

    out = np.empty((B, S, O), dtype=np.float32)
    for c in range(NCORES):
        b, og = c // OG, c % OG
        out[b, :, og * O_SH : (og + 1) * O_SH] = res.results[c]["out"]
    return out
